# revision 5
# baseline (speedup 1.0000x reference)
"""Trainium2 Bass kernel for nn_MultiHeadAttentionBlock (B=2, L=2048, D=1024, H=16).

Sharding: 8 cores = 2 batches x 4 head-groups (4 heads each), Megatron-style.
Each core computes q/k/v projections for its 4 heads (column-sharded weights),
RoPE, attention, and a partial output projection (row-sharded w_o). The host
sums the 4 partial outputs per batch (the "all-reduce").

v3: single fully-overlapped pipeline, ACT(exp)-paced.
  - Prelude (~20us): K projection (kT streamed via ring-6), rope on ACT/DVE,
    then Q projection for t1=0 only (qT fetched as a 512-column block), so
    attention block 0 starts as early as possible.
  - 8 attention blocks x 16 steps. Per-step PE emission puts the scores
    matmuls FIRST so the Activation engine's exp stream (the per-core floor,
    ~1.1us/step) never starves behind attn@V / projection matmuls.
  - Q projections for t1=1..3 run inside odd blocks (psum borrowed from the
    aux ring), rope evac on DVE, repack on GpSimd. V projection is JIT inside
    block 0 from column-sliced vT DMA. Output projection runs inside even
    blocks. This keeps the PE dense (no >3.4us idle => HAM stays at 2.4GHz).
  - Mask is pre-duplicated on the host to [L, 4, 2, 512] so the mask multiply
    is a plain stride-1 fp16 tensor_tensor on DVE (2x mode) instead of a
    broadcast access pattern; every 4th step's multiply runs on GpSimd.
  - attn@V stays "flipped" (lhsT = P chunks, rhs = [V|1]) with token-major
    [128q, 65] accumulators; softmax normalization is a per-partition
    reciprocal+scale; a PE transpose returns O to feature-major for w_o.
"""

import contextlib
import sys

import numpy as np

sys.path.insert(0, "/opt/trn_rl_repo")

import ml_dtypes  # noqa: E402

import concourse.bass as bass  # noqa: E402
import concourse.tile as tile  # noqa: E402
from concourse import bacc, mybir  # noqa: E402
from concourse.bass import ts  # noqa: E402

F32 = mybir.dt.float32
FP16 = mybir.dt.float16
AF = mybir.ActivationFunctionType

B, L, D, H = 2, 2048, 1024, 16
DK = D // H          # 64
HG = 4               # heads per core
DH = HG * DK         # 256 features per core
N_CORES = 8
KC = D // 128        # 8 contraction chunks for projections
T1C = 4              # number of 512-wide query chunks
T2C = 16             # number of 128-wide key chunks
NKX = 6              # kT streaming ring


def build_kernel():
    nc = bacc.Bacc(
        "TRN2",
        target_bir_lowering=False,
        debug=False,
        enable_asserts=False,
        num_devices=N_CORES,
    )

    qT = nc.dram_tensor("qT", [D, L], FP16, kind="ExternalInput").ap()
    kT = nc.dram_tensor("kT", [D, L], FP16, kind="ExternalInput").ap()
    vT = nc.dram_tensor("vT", [D, L], FP16, kind="ExternalInput").ap()
    wq = nc.dram_tensor("wq", [D, DH], FP16, kind="ExternalInput").ap()
    wk = nc.dram_tensor("wk", [D, DH], FP16, kind="ExternalInput").ap()
    wv = nc.dram_tensor("wv", [D, DH], FP16, kind="ExternalInput").ap()
    wo = nc.dram_tensor("wo", [DH, D], FP16, kind="ExternalInput").ap()
    cosT = nc.dram_tensor("cosT", [128, L], FP16, kind="ExternalInput").ap()
    sinT = nc.dram_tensor("sinT", [128, L], FP16, kind="ExternalInput").ap()
    maskT = nc.dram_tensor("maskT", [L, T1C, 2, 512], FP16,
                           kind="ExternalInput").ap()
    ident = nc.dram_tensor("ident", [128, 128], F32, kind="ExternalInput").ap()
    out = nc.dram_tensor("out", [L, D], FP16, kind="ExternalOutput").ap()

    qT_c = qT.rearrange("(c p) n -> p c n", p=128)        # [128, 8, 2048]
    kT_c = kT.rearrange("(c p) n -> p c n", p=128)
    vT_c = vT.rearrange("(c p) n -> p c n", p=128)
    wq_c = wq.rearrange("(c p) n -> p c n", p=128)        # [128, 8, 256]
    wk_c = wk.rearrange("(c p) n -> p c n", p=128)
    wv_c = wv.rearrange("(c p) n -> p c n", p=128)
    wo_c = wo.rearrange("(c p) n -> p c n", p=128)        # [128, 2, 1024]
    maskT_r = maskT.rearrange("(c p) t j n -> p c t j n", p=128)
    out_c = out.rearrange("(t p) n -> p t n", p=128)      # [128, 16, 1024]

    with tile.TileContext(nc) as tc, contextlib.ExitStack() as top:
        persist = top.enter_context(tc.tile_pool(name="persist", bufs=1))
        mpool = top.enter_context(tc.tile_pool(name="mask", bufs=1))
        pmpool = top.enter_context(tc.tile_pool(name="pm", bufs=7))
        pepool = top.enter_context(tc.tile_pool(name="pex", bufs=3))
        oqpool = top.enter_context(tc.tile_pool(name="oq", bufs=4))
        ospool = top.enter_context(tc.tile_pool(name="ostage", bufs=3))
        smallp = top.enter_context(tc.tile_pool(name="small", bufs=4))
        kxpool = top.enter_context(tc.tile_pool(name="kx", bufs=1))
        qcbpool = top.enter_context(tc.tile_pool(name="qcb", bufs=1))
        vcbpool = top.enter_context(tc.tile_pool(name="vcb", bufs=1))
        rtpool = top.enter_context(tc.tile_pool(name="rt", bufs=1))
        prpool = top.enter_context(tc.tile_pool(name="prp", bufs=1))

        # ---- persistent SBUF ------------------------------------------------
        KT_hc = [persist.tile([128, L], FP16, name=f"KThc{p}", tag=f"KThc{p}")
                 for p in range(2)]
        QT_hc = [persist.tile([128, L], FP16, name=f"QThc{p}", tag=f"QThc{p}")
                 for p in range(2)]
        V_all = persist.tile([128, T2C, HG, DK + 1], FP16, name="V_all",
                             tag="V_all")
        OT_sb = [persist.tile([128, L], FP16, name=f"OTsb{p}", tag=f"OT{p}")
                 for p in range(2)]
        wk_sb = persist.tile([128, KC, DH], FP16, name="wk_sb", tag="wk")
        wq_sb = persist.tile([128, KC, DH], FP16, name="wq_sb", tag="wq")
        wv_sb = persist.tile([128, KC, DH], FP16, name="wv_sb", tag="wv")
        wo_sb = persist.tile([128, 2, D], FP16, name="wo_sb", tag="wo")
        cos_h = persist.tile([128, L], FP16, name="cos_h", tag="cos")
        sin_h = persist.tile([128, L], FP16, name="sin_h", tag="sin")
        id_sb = persist.tile([128, 128], F32, name="id_sb", tag="ident")

        kxt = {}
        qcb = {}
        vcb = {}
        mt = {}

        def kx_dma(kk):
            t = kxpool.tile([128, L], FP16, name=f"kx{kk}",
                            tag=f"x{kk % NKX}")
            nc.sync.dma_start(t[:], kT_c[:, kk, :])
            kxt[kk] = t

        def qcb_dma(t1):
            t = qcbpool.tile([128, KC, 512], FP16, name=f"qcb{t1}",
                             tag=f"q{t1 % 2}")
            nc.sync.dma_start(t[:], qT_c[:, :, ts(t1, 512)])
            qcb[t1] = t

        def vcb_dma(c):
            t = vcbpool.tile([128, KC, 512], FP16, name=f"vcb{c}",
                             tag=f"v{c % 3}")
            nc.sync.dma_start(t[:], vT_c[:, :, ts(c, 512)])
            vcb[c] = t

        def mask_dma(t1, hf):
            t = mpool.tile([128, 8, 2, 512], FP16, name=f"mt{t1}{hf}",
                           tag=f"mh{hf}")
            nc.sync.dma_start(t[:], maskT_r[:, ts(hf, 8), t1, :, :])
            mt[(t1, hf)] = t

        # ---- DMA preamble (order = priority) --------------------------------
        nc.sync.dma_start(wk_sb[:], wk_c)
        for kk in range(NKX):
            kx_dma(kk)
        nc.sync.dma_start(wq_sb[:], wq_c)
        qcb_dma(0)
        nc.sync.dma_start(cos_h[:], cosT)
        nc.sync.dma_start(sin_h[:], sinT)
        mask_dma(0, 0)
        nc.vector.memset(V_all[:, :, :, DK:DK + 1], 1.0)
        nc.sync.dma_start(wv_sb[:], wv_c)
        vcb_dma(0)
        mask_dma(0, 1)
        nc.sync.dma_start(id_sb[:], ident)

        # ---- rope helpers ---------------------------------------------------
        def rope(ps0, ps1, csl, w, dst0, dst1, act_evac, wtag):
            # dst0 = x0*c - x1*s ; dst1 = x1*c + x0*s
            x0f = rtpool.tile([128, w], FP16, name="x0f", tag=f"x0f{wtag}")
            x1f = rtpool.tile([128, w], FP16, name="x1f", tag=f"x1f{wtag}")
            if act_evac:
                nc.scalar.copy(x0f[:], ps0[:])
                nc.scalar.copy(x1f[:], ps1[:])
            else:
                nc.vector.tensor_copy(x0f[:], ps0[:])
                nc.vector.tensor_copy(x1f[:], ps1[:])
            c = cos_h[:, csl]
            s = sin_h[:, csl]
            x0c = rtpool.tile([128, w], FP16, name="x0c", tag=f"x0c{wtag}")
            x1s = rtpool.tile([128, w], FP16, name="x1s", tag=f"x1s{wtag}")
            x1c = rtpool.tile([128, w], FP16, name="x1c", tag=f"x1c{wtag}")
            x0s = rtpool.tile([128, w], FP16, name="x0s", tag=f"x0s{wtag}")
            nc.vector.tensor_mul(x0c[:], x0f[:], c)
            nc.vector.tensor_mul(x1s[:], x1f[:], s)
            nc.vector.tensor_mul(x1c[:], x1f[:], c)
            nc.vector.tensor_mul(x0s[:], x0f[:], s)
            nc.vector.tensor_sub(dst0[:], x0c[:], x1s[:])
            nc.vector.tensor_add(dst1[:], x1c[:], x0s[:])

        def repack(src0, src1, dst, sl, eng, hhs=range(HG)):
            # head-contiguous: dst[p][64j+32*half+..] <- src[half][32hh+..]
            for hh in hhs:
                p_, j_ = divmod(hh, 2)
                for half, src in enumerate((src0, src1)):
                    eng.tensor_copy(
                        dst[p_][64 * j_ + 32 * half:
                                64 * j_ + 32 * half + 32, sl],
                        src[32 * hh:32 * hh + 32, :])

        # ---- prelude: K projection + rope, Q(t1=0) projection + rope --------
        with tc.tile_pool(name="pp", bufs=1, space="PSUM") as pp:
            ps = [pp.tile([128, 1024], F32, name=f"ps{q}", tag=f"ps{q}")
                  for q in range(4)]  # index fh*2+th
            for kk in range(KC):
                for fh in range(2):
                    for th in range(2):
                        p_ = ps[fh * 2 + th]
                        for n in range(2):
                            nc.tensor.matmul(
                                p_[:, ts(n, 512)],
                                lhsT=wk_sb[:, kk, ts(fh, 128)],
                                rhs=kxt[kk][:, th * 1024 + n * 512:
                                            th * 1024 + (n + 1) * 512],
                                start=(kk == 0),
                                stop=(kk == KC - 1),
                            )
                if kk + NKX < KC:
                    kx_dma(kk + NKX)
            # late prefetches: stream behind the preamble
            vcb_dma(1)
            qcb_dma(1)
            vcb_dma(2)
            nc.sync.dma_start(wo_sb[:], wo_c)

            KT_sb = [prpool.tile([128, L], FP16, name=f"KTsb{h}",
                                 tag=f"pr{h}") for h in range(2)]
            for th in range(2):
                rope(ps[th], ps[2 + th], ts(th, 1024), 1024,
                     KT_sb[0][:, ts(th, 1024)], KT_sb[1][:, ts(th, 1024)],
                     act_evac=True, wtag="k")

            # Q projection for t1=0 (reuses ps0/ps1 banks)
            qps = [pp.tile([128, 512], F32, name=f"qps{fh}", tag=f"ps{fh}")
                   for fh in range(2)]
            for kk in range(KC):
                for fh in range(2):
                    nc.tensor.matmul(
                        qps[fh][:],
                        lhsT=wq_sb[:, kk, ts(fh, 128)],
                        rhs=qcb[0][:, kk, :],
                        start=(kk == 0),
                        stop=(kk == KC - 1),
                    )
            QT_sb = [prpool.tile([128, 512], FP16, name=f"QTsb{h}",
                                 tag=f"qr{h}") for h in range(2)]
            rope(qps[0], qps[1], ts(0, 512), 512, QT_sb[0][:], QT_sb[1][:],
                 act_evac=True, wtag="q")
            repack(KT_sb[0], KT_sb[1], KT_hc, slice(0, L), nc.vector)
            repack(QT_sb[0], QT_sb[1], QT_hc, slice(0, 512), nc.vector)

        # ---- attention: 8 blocks x 16 steps ---------------------------------
        with tc.tile_pool(name="att_psum", bufs=1, space="PSUM") as apsum, \
             tc.tile_pool(name="aux_psum", bufs=2, space="PSUM") as aux:

            def scores_mm(b, i):
                t1, p = b // 2, b % 2
                psc = apsum.tile([128, 1024], F32, name="psc", tag="psc",
                                 bufs=2)
                for j in range(2):
                    nc.tensor.matmul(
                        psc[:, ts(j, 512)],
                        lhsT=KT_hc[p][ts(j, 64), ts(i, 128)],
                        rhs=QT_hc[p][ts(j, 64), ts(t1, 512)],
                        start=True, stop=True,
                        tile_position=(64 * j, 0),
                    )
                return psc

            def exp_mask(b, i, psc):
                t1 = b // 2
                pex = pepool.tile([128, 1024], FP16, name="pex", tag="pex")
                nc.scalar.activation(pex[:], psc[:], AF.Exp)
                pm = pmpool.tile([128, 1024], FP16, name="pm", tag="pm")
                eng = nc.gpsimd if i % 4 == 3 else nc.vector
                eng.tensor_mul(pm[:], pex[:], mt[(t1, i // 8)][:, i % 8, :, :])
                return pm

            def attnv_mm(b, i2, pm, accs):
                p = b % 2
                for j in range(2):
                    for qc in range(4):
                        nc.tensor.matmul(
                            accs[j][:, qc * 128:qc * 128 + DK + 1],
                            lhsT=pm[:, j * 512 + qc * 128:
                                    j * 512 + (qc + 1) * 128],
                            rhs=V_all[:, i2, 2 * p + j, :],
                            start=(i2 == 0 and qc == 0),
                            stop=(i2 == T2C - 1 and qc == 3),
                        )

            def vproj(tt):
                pv = aux.tile([128, 512], F32, name="pv", tag="aux")
                for kk in range(KC):
                    nc.tensor.matmul(
                        pv[:, 0:DH],
                        lhsT=vcb[tt // 4][:, kk, ts(tt % 4, 128)],
                        rhs=wv_sb[:, kk, :],
                        start=(kk == 0),
                        stop=(kk == KC - 1),
                    )
                nc.vector.tensor_copy(
                    V_all[:, tt, :, 0:DK],
                    pv[:, 0:DH].rearrange("p (h d) -> p h d", h=HG))

            def normalize(b, accs):
                oqs = []
                for qc in range(4):
                    oq = oqpool.tile([128, 128], F32, name="oq", tag="oq")
                    for j in range(2):
                        rc = smallp.tile([128, 1], F32, name="rc", tag="rc")
                        nc.vector.reciprocal_approx_fast(
                            rc[:], accs[j][:, qc * 128 + DK:qc * 128 + DK + 1])
                        nc.vector.tensor_scalar_mul(
                            oq[:, ts(j, DK)],
                            accs[j][:, qc * 128:qc * 128 + DK], rc[:])
                    oqs.append(oq)
                return oqs

            def transpose_evac(b, oqs):
                t1, p = b // 2, b % 2
                tp = aux.tile([128, 512], F32, name="tp", tag="aux")
                for qc in range(4):
                    nc.tensor.matmul(
                        tp[:, ts(qc, 128)], lhsT=oqs[qc][:], rhs=id_sb[:],
                        is_transpose=True, start=True, stop=True,
                    )
                nc.vector.tensor_copy(OT_sb[p][:, ts(t1, 512)], tp[:])

            def outproj_chunk(t1o, ci, psc_bank=False):
                # ci in 0..7: t-tile = 4*t1o + ci//2, column half ci%2.
                t = 4 * t1o + ci // 2
                jj = ci % 2
                if psc_bank:
                    po = apsum.tile([128, 1024], F32, name="po2", tag="psc",
                                    bufs=2)[:, 0:512]
                else:
                    po = aux.tile([128, 512], F32, name="po", tag="aux")[:]
                for pp_ in range(2):
                    nc.tensor.matmul(
                        po,
                        lhsT=OT_sb[pp_][:, ts(t, 128)],
                        rhs=wo_sb[:, pp_, ts(jj, 512)],
                        start=(pp_ == 0),
                        stop=(pp_ == 1),
                    )
                ob = ospool.tile([128, 512], FP16, name="ob", tag="ob")
                nc.vector.tensor_copy(ob[:], po)
                nc.sync.dma_start(out_c[:, t, ts(jj, 512)], ob[:])

            # Q projection for t1 = 1..3, spread across the odd block b=2*t1-1
            qp_ps = {}

            def qproj_sweep(t1, idx):
                # idx 0..7: fh = idx//4, kk pair = 2*(idx%4), 2*(idx%4)+1
                fh = idx // 4
                if idx % 4 == 0:
                    if fh == 0:
                        qp_ps[t1] = []
                    qp_ps[t1].append(
                        aux.tile([128, 512], F32, name=f"qp{t1}{fh}",
                                 tag="aux"))
                p_ = qp_ps[t1][fh]
                for kk in (2 * (idx % 4), 2 * (idx % 4) + 1):
                    nc.tensor.matmul(
                        p_[:],
                        lhsT=wq_sb[:, kk, ts(fh, 128)],
                        rhs=qcb[t1][:, kk, :],
                        start=(kk == 0),
                        stop=(kk == KC - 1),
                    )

            def qproj_rope(t1):
                QT_t = [prpool.tile([128, 512], FP16, name=f"QTt{h}",
                                    tag=f"qr{h}") for h in range(2)]
                rope(qp_ps[t1][0], qp_ps[t1][1], ts(t1, 512), 512,
                     QT_t[0][:], QT_t[1][:], act_evac=False, wtag="q")
                return QT_t

            pm_hist = {}      # (b, i) -> pm tile
            acc_hist = {}     # b -> accs
            oq_hist = {}      # b -> oq tiles
            qt_hist = {}      # t1 -> staged rope output
            TAIL = ((12, 13), (14, 15))
            for b in range(8):
                t1, p = b // 2, b % 2
                accs = [apsum.tile([128, 512], F32, name=f"acc{j}",
                                   tag=f"acc{j}") for j in range(2)]
                acc_hist[b] = accs
                for i in range(T2C):
                    # ---- DMA prefetch points --------------------------------
                    if b == 0 and i == 2:
                        qcb_dma(2)
                    if b == 0 and i == 7:
                        vcb_dma(3)
                    if b == 1 and i == 12:
                        qcb_dma(3)
                    if p == 1 and t1 + 1 < T1C and i == 8:
                        mask_dma(t1 + 1, 0)
                    if p == 0 and b >= 2 and i == 0:
                        mask_dma(t1, 1)
                    # ---- scores first: keeps ACT fed ------------------------
                    psc = scores_mm(b, i)
                    pm_hist[(b, i)] = exp_mask(b, i, psc)
                    # ---- previous block wrap-up -----------------------------
                    if b >= 1 and i < 2:
                        for i2 in TAIL[i]:
                            attnv_mm(b - 1, i2, pm_hist[(b - 1, i2)],
                                     acc_hist[b - 1])
                    if b >= 1 and i == 2:
                        oq_hist[b - 1] = normalize(b - 1, acc_hist[b - 1])
                        del acc_hist[b - 1]
                    if b >= 1 and i == 3:
                        transpose_evac(b - 1, oq_hist.pop(b - 1))
                    # ---- this block's attn@V (lag 4) ------------------------
                    if i >= 4:
                        attnv_mm(b, i - 4, pm_hist[(b, i - 4)], accs)
                    # ---- interleaves ----------------------------------------
                    if b == 0 and i >= 3:
                        vproj(i - 3)
                        if i >= 13:
                            vproj(i)
                    if p == 1 and t1 + 1 < T1C:
                        # Q projection for the next t1 inside this odd block
                        if 4 <= i < 12:
                            qproj_sweep(t1 + 1, i - 4)
                        elif i == 12:
                            qt_hist[t1 + 1] = qproj_rope(t1 + 1)
                        elif i in (13, 14):
                            qt = qt_hist[t1 + 1]
                            repack(qt[0], qt[1], QT_hc,
                                   slice((t1 + 1) * 512, (t1 + 2) * 512),
                                   nc.gpsimd,
                                   hhs=range(0, 2) if i == 13 else range(2, 4))
                    if p == 0 and b >= 2:
                        # output projection for t1o = t1 - 1 (8 chunks)
                        if i in (4, 5, 6, 7):
                            outproj_chunk(t1 - 1, i - 4)
                        elif i in (8, 10, 12, 14):
                            outproj_chunk(t1 - 1, 4 + (i - 8) // 2)

            # ---- tail: finish block 7, then t1=3 output projection ----------
            b = 7
            for i2 in TAIL[0] + TAIL[1]:
                attnv_mm(b, i2, pm_hist[(b, i2)], acc_hist[b])
            oqs = normalize(b, acc_hist[b])
            transpose_evac(b, oqs)
            for ci in range(8):
                outproj_chunk(3, ci, psc_bank=(ci % 2 == 1))

    nc.compile()
    return nc


def shard_inputs(q, k, v, mask, w_q, w_k, w_v, w_o):
    q = np.asarray(q, np.float32)
    k = np.asarray(k, np.float32)
    v = np.asarray(v, np.float32)
    w_q = np.asarray(w_q, np.float32)
    w_k = np.asarray(w_k, np.float32)
    w_v = np.asarray(w_v, np.float32)
    w_o = np.asarray(w_o, np.float32)
    mask = np.asarray(mask)

    qT = [np.ascontiguousarray(q[b].T).astype(np.float16) for b in range(B)]
    kT = [np.ascontiguousarray(k[b].T).astype(np.float16) for b in range(B)]
    vT = [np.ascontiguousarray(v[b].T).astype(np.float16) for b in range(B)]
    maskT_bf = np.ascontiguousarray(mask[0, 0].T).astype(np.float16)  # [k, q]
    md = maskT_bf.reshape(L, T1C, 512)
    mask_dup = np.ascontiguousarray(
        np.broadcast_to(md[:, :, None, :], (L, T1C, 2, 512))).astype(
            np.float16)

    inv = 1.0 / (10000.0 ** (np.arange(0, DK, 2) / DK))   # [32]
    t = np.arange(L)
    fr = np.outer(inv, t)                                 # [32, 2048]
    cos_tab = np.tile(np.cos(fr), (4, 1)).astype(np.float16)  # [128, 2048]
    sin_tab = np.tile(np.sin(fr), (4, 1)).astype(np.float16)
    ident = np.eye(128, dtype=np.float32)

    even = np.arange(0, DK, 2)
    odd = np.arange(1, DK, 2)
    scale = 1.0 / np.sqrt(DK)

    in_maps = []
    for core in range(N_CORES):
        b, g = divmod(core, N_CORES // B)
        hs = [HG * g + i for i in range(HG)]
        rows_qk = np.concatenate([h * DK + even for h in hs]
                                 + [h * DK + odd for h in hs])
        rows_v = np.concatenate([np.arange(h * DK, (h + 1) * DK) for h in hs])
        in_maps.append({
            "qT": qT[b],
            "kT": kT[b],
            "vT": vT[b],
            "wq": np.ascontiguousarray((w_q[rows_qk, :] * scale).T).astype(np.float16),
            "wk": np.ascontiguousarray(w_k[rows_qk, :].T).astype(np.float16),
            "wv": np.ascontiguousarray(w_v[rows_v, :].T).astype(np.float16),
            "wo": np.ascontiguousarray(w_o[:, rows_v].T).astype(np.float16),
            "cosT": cos_tab,
            "sinT": sin_tab,
            "maskT": mask_dup,
            "ident": ident,
        })
    return in_maps


_compiled = None


def _get_compiled():
    global _compiled
    if _compiled is None:
        _compiled = build_kernel()
    return _compiled


def kernel(q, k, v, mask, w_q, w_k, w_v, w_o, _trace=False, _trace_cores=None):
    from concourse.bass_utils import run_bass_kernel_spmd

    nc = _get_compiled()
    in_maps = shard_inputs(q, k, v, mask, w_q, w_k, w_v, w_o)
    res = run_bass_kernel_spmd(
        nc, in_maps, core_ids=list(range(N_CORES)),
        trace=_trace, trace_cores=_trace_cores,
    )
    out = np.zeros((B, L, D), np.float32)
    for core in range(N_CORES):
        out[core // (N_CORES // B)] += res.results[core]["out"].astype(np.float32)
    kernel._last_results = res
    return out


# revision 18
# speedup vs baseline: 1.1646x; 1.1646x over previous
"""Trainium2 Bass kernel for nn_MultiHeadAttentionBlock (B=2, L=2048, D=1024, H=16).

Sharding: 8 cores = 2 batches x 4 head-groups (4 heads each), Megatron-style.
Each core computes q/k/v projections for its 4 heads (column-sharded weights),
RoPE, attention, and a partial output projection (row-sharded w_o). The host
sums the 4 partial outputs per batch (the "all-reduce").

v3: single fully-overlapped pipeline, ACT(exp)-paced.
  - Prelude (~20us): K projection (kT streamed via ring-6), rope on ACT/DVE,
    then Q projection for t1=0 only (qT fetched as a 512-column block), so
    attention block 0 starts as early as possible.
  - 8 attention blocks x 16 steps. Per-step PE emission puts the scores
    matmuls FIRST so the Activation engine's exp stream (the per-core floor,
    ~1.1us/step) never starves behind attn@V / projection matmuls.
  - Q projections for t1=1..3 run inside odd blocks (psum borrowed from the
    aux ring), rope evac on DVE, repack on GpSimd. V projection is JIT inside
    block 0 from column-sliced vT DMA. Output projection runs inside even
    blocks. This keeps the PE dense (no >3.4us idle => HAM stays at 2.4GHz).
  - Mask is pre-duplicated on the host to [L, 4, 2, 512] so the mask multiply
    is a plain stride-1 fp16 tensor_tensor on DVE (2x mode) instead of a
    broadcast access pattern; every 4th step's multiply runs on GpSimd.
  - attn@V stays "flipped" (lhsT = P chunks, rhs = [V|1]) with token-major
    [128q, 65] accumulators; softmax normalization is a per-partition
    reciprocal+scale; a PE transpose returns O to feature-major for w_o.
"""

import contextlib
import sys

import numpy as np

sys.path.insert(0, "/opt/trn_rl_repo")

import ml_dtypes  # noqa: E402

import concourse.bass as bass  # noqa: E402
import concourse.tile as tile  # noqa: E402
from concourse import bacc, mybir  # noqa: E402
from concourse.bass import ts  # noqa: E402

F32 = mybir.dt.float32
FP16 = mybir.dt.float16
AF = mybir.ActivationFunctionType

B, L, D, H = 2, 2048, 1024, 16
DK = D // H          # 64
HG = 4               # heads per core
DH = HG * DK         # 256 features per core
N_CORES = 8
KC = D // 128        # 8 contraction chunks for projections
T1C = 4              # number of 512-wide query chunks
T2C = 16             # number of 128-wide key chunks
NKX = 8              # kT chunks all DMA'd up front


def build_kernel():
    nc = bacc.Bacc(
        "TRN2",
        target_bir_lowering=False,
        debug=False,
        enable_asserts=False,
        num_devices=N_CORES,
    )

    qT = nc.dram_tensor("qT", [D, L], FP16, kind="ExternalInput").ap()
    kT = nc.dram_tensor("kT", [D, L], FP16, kind="ExternalInput").ap()
    vT = nc.dram_tensor("vT", [D, L], FP16, kind="ExternalInput").ap()
    wq = nc.dram_tensor("wq", [D, DH], FP16, kind="ExternalInput").ap()
    wk = nc.dram_tensor("wk", [D, DH], FP16, kind="ExternalInput").ap()
    wv = nc.dram_tensor("wv", [D, DH], FP16, kind="ExternalInput").ap()
    wo = nc.dram_tensor("wo", [DH, D], FP16, kind="ExternalInput").ap()
    cosT = nc.dram_tensor("cosT", [128, L], FP16, kind="ExternalInput").ap()
    sinT = nc.dram_tensor("sinT", [128, L], FP16, kind="ExternalInput").ap()
    maskT = nc.dram_tensor("maskT", [L, T1C, 2, 512], FP16,
                           kind="ExternalInput").ap()
    ident = nc.dram_tensor("ident", [128, 128], F32, kind="ExternalInput").ap()
    out = nc.dram_tensor("out", [L, D], FP16, kind="ExternalOutput").ap()

    qT_c = qT.rearrange("(c p) n -> p c n", p=128)        # [128, 8, 2048]
    kT_c = kT.rearrange("(c p) n -> p c n", p=128)
    vT_c = vT.rearrange("(c p) n -> p c n", p=128)
    wq_c = wq.rearrange("(c p) n -> p c n", p=128)        # [128, 8, 256]
    wk_c = wk.rearrange("(c p) n -> p c n", p=128)
    wv_c = wv.rearrange("(c p) n -> p c n", p=128)
    wo_c = wo.rearrange("(c p) n -> p c n", p=128)        # [128, 2, 1024]
    maskT_r = maskT.rearrange("(c p) t j n -> p c t j n", p=128)
    out_c = out.rearrange("(t p) n -> p t n", p=128)      # [128, 16, 1024]

    with tile.TileContext(nc) as tc, contextlib.ExitStack() as top:
        persist = top.enter_context(tc.tile_pool(name="persist", bufs=1))
        mpool = top.enter_context(tc.tile_pool(name="mask", bufs=1))
        pmpool = top.enter_context(tc.tile_pool(name="pm", bufs=7))
        pepool = top.enter_context(tc.tile_pool(name="pex", bufs=3))
        oqpool = top.enter_context(tc.tile_pool(name="oq", bufs=4))
        ospool = top.enter_context(tc.tile_pool(name="ostage", bufs=3))
        smallp = top.enter_context(tc.tile_pool(name="small", bufs=4))
        kxpool = top.enter_context(tc.tile_pool(name="kx", bufs=1))
        qcbpool = top.enter_context(tc.tile_pool(name="qcb", bufs=1))
        vcbpool = top.enter_context(tc.tile_pool(name="vcb", bufs=1))
        rtpool = top.enter_context(tc.tile_pool(name="rt", bufs=1))
        prpool = top.enter_context(tc.tile_pool(name="prp", bufs=1))

        # ---- persistent SBUF ------------------------------------------------
        KT_hc = [persist.tile([128, L], FP16, name=f"KThc{p}", tag=f"KThc{p}")
                 for p in range(2)]
        QT_hc = [persist.tile([128, L], FP16, name=f"QThc{p}", tag=f"QThc{p}")
                 for p in range(2)]
        V_all = persist.tile([128, T2C, HG, DK + 1], FP16, name="V_all",
                             tag="V_all")
        OT_sb = [persist.tile([128, L], FP16, name=f"OTsb{p}", tag=f"OT{p}")
                 for p in range(2)]
        wk_sb = persist.tile([128, KC, DH], FP16, name="wk_sb", tag="wk")
        wq_sb = persist.tile([128, KC, DH], FP16, name="wq_sb", tag="wq")
        wv_sb = persist.tile([128, KC, DH], FP16, name="wv_sb", tag="wv")
        wo_sb = persist.tile([128, 2, D], FP16, name="wo_sb", tag="wo")
        cos_h = persist.tile([128, L], FP16, name="cos_h", tag="cos")
        sin_h = persist.tile([128, L], FP16, name="sin_h", tag="sin")
        id_sb = persist.tile([128, 128], F32, name="id_sb", tag="ident")

        kxt = {}
        qcb = {}
        vcb = {}
        mt = {}

        def kx_dma(kk):
            t = kxpool.tile([128, L], FP16, name=f"kx{kk}",
                            tag=f"x{kk % NKX}")
            nc.sync.dma_start(t[:], kT_c[:, kk, :])
            kxt[kk] = t

        def qcb_dma(t1):
            t = qcbpool.tile([128, KC, 512], FP16, name=f"qcb{t1}",
                             tag=f"q{t1 % 2}")
            nc.sync.dma_start(t[:], qT_c[:, :, ts(t1, 512)])
            qcb[t1] = t

        def vcb_dma(c):
            t = vcbpool.tile([128, KC, 512], FP16, name=f"vcb{c}",
                             tag=f"v{c % 2}")
            nc.sync.dma_start(t[:], vT_c[:, :, ts(c, 512)])
            vcb[c] = t

        def mask_dma(t1, hf):
            t = mpool.tile([128, 8, 2, 512], FP16, name=f"mt{t1}{hf}",
                           tag=f"mh{hf}")
            nc.sync.dma_start(t[:], maskT_r[:, ts(hf, 8), t1, :, :])
            mt[(t1, hf)] = t

        # ---- DMA preamble (order = priority; queues drain FIFO) -------------
        nc.sync.dma_start(wq_sb[:], wq_c)
        qcb_dma(0)
        nc.sync.dma_start(wk_sb[:], wk_c)
        for kk in range(NKX):
            kx_dma(kk)
        nc.sync.dma_start(cos_h[:], cosT)
        nc.sync.dma_start(sin_h[:], sinT)
        mask_dma(0, 0)
        nc.vector.memset(V_all[:, :, :, DK:DK + 1], 1.0)
        nc.sync.dma_start(wv_sb[:], wv_c)
        vcb_dma(0)
        mask_dma(0, 1)
        vcb_dma(1)
        nc.sync.dma_start(id_sb[:], ident)

        # ---- rope helpers ---------------------------------------------------
        class Rope:
            """Granular rope: evac -> muls -> combine, so the pieces can be
            spread across pipeline steps.  dst0 = x0*c - x1*s ; dst1 =
            x1*c + x0*s."""

            def __init__(self, ps0, ps1, csl, w, wtag):
                self.ps0, self.ps1 = ps0, ps1
                self.c = cos_h[:, csl]
                self.s = sin_h[:, csl]
                self.t = {nm: rtpool.tile([128, w], FP16, name=nm,
                                          tag=f"{nm}{wtag}")
                          for nm in ("x0f", "x1f", "x0c", "x1s", "x1c", "x0s")}

            def evac0(self, act):
                if act:
                    nc.scalar.copy(self.t["x0f"][:], self.ps0[:])
                else:
                    nc.vector.tensor_copy(self.t["x0f"][:], self.ps0[:])

            def evac1(self, act):
                if act:
                    nc.scalar.copy(self.t["x1f"][:], self.ps1[:])
                else:
                    nc.vector.tensor_copy(self.t["x1f"][:], self.ps1[:])

            def muls0(self):
                nc.vector.tensor_mul(self.t["x0c"][:], self.t["x0f"][:],
                                     self.c)
                nc.vector.tensor_mul(self.t["x0s"][:], self.t["x0f"][:],
                                     self.s)

            def muls1(self):
                nc.vector.tensor_mul(self.t["x1c"][:], self.t["x1f"][:],
                                     self.c)
                nc.vector.tensor_mul(self.t["x1s"][:], self.t["x1f"][:],
                                     self.s)

            def combine(self, dst0, dst1):
                nc.vector.tensor_sub(dst0[:], self.t["x0c"][:],
                                     self.t["x1s"][:])
                nc.vector.tensor_add(dst1[:], self.t["x1c"][:],
                                     self.t["x0s"][:])

        def rope(ps0, ps1, csl, w, dst0, dst1, act_evac, wtag):
            r = Rope(ps0, ps1, csl, w, wtag)
            r.evac0(act_evac)
            r.evac1(act_evac)
            r.muls0()
            r.muls1()
            r.combine(dst0, dst1)

        def repack(src0, src1, dst, sl, eng, hhs=range(HG), ssl=slice(None)):
            # head-contiguous: dst[p][64j+32*half+..] <- src[half][32hh+..]
            for hh in hhs:
                p_, j_ = divmod(hh, 2)
                for half, src in enumerate((src0, src1)):
                    eng.tensor_copy(
                        dst[p_][64 * j_ + 32 * half:
                                64 * j_ + 32 * half + 32, sl],
                        src[32 * hh:32 * hh + 32, ssl])

        # ---- prelude: th-staged K projection + JIT Q(t1=0) ------------------
        # PE order: K-th0 sweep, Q(t1=0) sweep, K-th1 sweep.  ACT evacuates
        # th0 psums while Q matmuls run; DVE ropes + repacks th0 and Q so
        # attention block 0 can start; th1 rope/repack trails into block 0
        # (its keys are only needed from step 8).
        KT_sb = [prpool.tile([128, L], FP16, name=f"KTsb{h}",
                             tag=f"pr{h}") for h in range(2)]
        QT_sb = [prpool.tile([128, 512], FP16, name=f"QTsb{h}",
                             tag=f"qr{h}") for h in range(2)]
        with tc.tile_pool(name="pp", bufs=1, space="PSUM") as pp:
            ps = {}
            for th in range(2):
                for fh in range(2):
                    ps[(th, fh)] = pp.tile([128, 1024], F32,
                                           name=f"ps{th}{fh}",
                                           tag=f"ps{th}{fh}")

            def kproj_sweep(th):
                for kk in range(KC):
                    for fh in range(2):
                        for n in range(2):
                            nc.tensor.matmul(
                                ps[(th, fh)][:, ts(n, 512)],
                                lhsT=wk_sb[:, kk, ts(fh, 128)],
                                rhs=kxt[kk][:, th * 1024 + n * 512:
                                            th * 1024 + (n + 1) * 512],
                                start=(kk == 0),
                                stop=(kk == KC - 1),
                            )

            kproj_sweep(0)
            # th0 rope (ACT evac) emitted now; runs while Q/th1 matmuls go
            rk0 = Rope(ps[(0, 0)], ps[(0, 1)], ts(0, 1024), 1024, "k")
            rk0.evac0(True)
            rk0.evac1(True)

            # Q projection for t1=0 (reuses th0 banks once ACT evacuated)
            qps = [pp.tile([128, 512], F32, name=f"qps{fh}",
                           tag=f"ps0{fh}") for fh in range(2)]
            for kk in range(KC):
                for fh in range(2):
                    nc.tensor.matmul(
                        qps[fh][:],
                        lhsT=wq_sb[:, kk, ts(fh, 128)],
                        rhs=qcb[0][:, kk, :],
                        start=(kk == 0),
                        stop=(kk == KC - 1),
                    )
            kproj_sweep(1)
            qcb_dma(1)
            nc.sync.dma_start(wo_sb[:], wo_c)

            # DVE: th0 rope -> th0 repack (pair0 first) -> Q rope -> Q repack
            rk0.muls0()
            rk0.muls1()
            rk0.combine(KT_sb[0][:, 0:1024], KT_sb[1][:, 0:1024])
            repack(KT_sb[0], KT_sb[1], KT_hc, slice(0, 1024), nc.vector,
                   ssl=slice(0, 1024))
            rq0 = Rope(qps[0], qps[1], ts(0, 512), 512, "q")
            rq0.evac0(True)
            rq0.evac1(True)
            rq0.muls0()
            rq0.muls1()
            rq0.combine(QT_sb[0][:], QT_sb[1][:])
            repack(QT_sb[0], QT_sb[1], QT_hc, slice(0, 512), nc.vector)
            # th1 rope on DVE now; its repack is spread into block 0
            rk1 = Rope(ps[(1, 0)], ps[(1, 1)], ts(1, 1024), 1024, "k")
            rk1.evac0(True)
            rk1.evac1(True)
            rk1.muls0()
            rk1.muls1()
            rk1.combine(KT_sb[0][:, 1024:2048], KT_sb[1][:, 1024:2048])

        # ---- attention: 8 blocks x 16 steps ---------------------------------
        with tc.tile_pool(name="att_psum", bufs=1, space="PSUM") as apsum, \
             tc.tile_pool(name="aux_psum", bufs=2, space="PSUM") as aux:

            def scores_mm(b, i):
                t1, p = b // 2, b % 2
                psc = apsum.tile([128, 1024], F32, name="psc", tag="psc",
                                 bufs=2)
                for j in range(2):
                    nc.tensor.matmul(
                        psc[:, ts(j, 512)],
                        lhsT=KT_hc[p][ts(j, 64), ts(i, 128)],
                        rhs=QT_hc[p][ts(j, 64), ts(t1, 512)],
                        start=True, stop=True,
                        tile_position=(64 * j, 0),
                    )
                return psc

            def exp_mask(b, i, psc):
                t1 = b // 2
                pex = pepool.tile([128, 1024], FP16, name="pex", tag="pex")
                nc.scalar.activation(pex[:], psc[:], AF.Exp)
                pm = pmpool.tile([128, 1024], FP16, name="pm", tag="pm")
                # block 0's DVE also carries the th1 K-repack + vproj evacs,
                # so it sheds half the mask multiplies to GpSimd there
                on_pool = (i % 2 == 1) if b == 0 else (i % 4 == 3)
                eng = nc.gpsimd if on_pool else nc.vector
                eng.tensor_mul(pm[:], pex[:], mt[(t1, i // 8)][:, i % 8, :, :])
                return pm

            def attnv_mm(b, i2, pm, accs):
                p = b % 2
                for j in range(2):
                    for qc in range(4):
                        nc.tensor.matmul(
                            accs[j][:, qc * 128:qc * 128 + DK + 1],
                            lhsT=pm[:, j * 512 + qc * 128:
                                    j * 512 + (qc + 1) * 128],
                            rhs=V_all[:, i2, 2 * p + j, :],
                            start=(i2 == 0 and qc == 0),
                            stop=(i2 == T2C - 1 and qc == 3),
                        )

            def vproj(tt):
                pv = aux.tile([128, 512], F32, name="pv", tag="aux")
                for kk in range(KC):
                    nc.tensor.matmul(
                        pv[:, 0:DH],
                        lhsT=vcb[tt // 4][:, kk, ts(tt % 4, 128)],
                        rhs=wv_sb[:, kk, :],
                        start=(kk == 0),
                        stop=(kk == KC - 1),
                    )
                nc.vector.tensor_copy(
                    V_all[:, tt, :, 0:DK],
                    pv[:, 0:DH].rearrange("p (h d) -> p h d", h=HG))

            def normalize(b, accs):
                oqs = []
                for qc in range(4):
                    oq = oqpool.tile([128, 128], F32, name="oq", tag="oq")
                    for j in range(2):
                        rc = smallp.tile([128, 1], F32, name="rc", tag="rc")
                        nc.vector.reciprocal_approx_fast(
                            rc[:], accs[j][:, qc * 128 + DK:qc * 128 + DK + 1])
                        nc.vector.tensor_scalar_mul(
                            oq[:, ts(j, DK)],
                            accs[j][:, qc * 128:qc * 128 + DK], rc[:])
                    oqs.append(oq)
                return oqs

            def transpose_evac(b, oqs):
                t1, p = b // 2, b % 2
                tp = aux.tile([128, 512], F32, name="tp", tag="aux")
                for qc in range(4):
                    nc.tensor.matmul(
                        tp[:, ts(qc, 128)], lhsT=oqs[qc][:], rhs=id_sb[:],
                        is_transpose=True, start=True, stop=True,
                    )
                nc.vector.tensor_copy(OT_sb[p][:, ts(t1, 512)], tp[:])

            def outproj_chunk(t1o, ci, psc_bank=False):
                # ci in 0..7: t-tile = 4*t1o + ci//2, column half ci%2.
                t = 4 * t1o + ci // 2
                jj = ci % 2
                if psc_bank:
                    po = apsum.tile([128, 1024], F32, name="po2", tag="psc",
                                    bufs=2)[:, 0:512]
                else:
                    po = aux.tile([128, 512], F32, name="po", tag="aux")[:]
                for pp_ in range(2):
                    nc.tensor.matmul(
                        po,
                        lhsT=OT_sb[pp_][:, ts(t, 128)],
                        rhs=wo_sb[:, pp_, ts(jj, 512)],
                        start=(pp_ == 0),
                        stop=(pp_ == 1),
                    )
                ob = ospool.tile([128, 512], FP16, name="ob", tag="ob")
                nc.vector.tensor_copy(ob[:], po)
                nc.sync.dma_start(out_c[:, t, ts(jj, 512)], ob[:])

            # Q projection for t1 = 1..3, spread across the odd block b=2*t1-1
            qp_ps = {}
            qt_rope = {}
            qt_hist = {}

            def qproj_sweep(t1, idx):
                # idx 0..7: fh = idx//4, kk pair = 2*(idx%4), 2*(idx%4)+1
                fh = idx // 4
                if idx % 4 == 0:
                    if fh == 0:
                        qp_ps[t1] = []
                    qp_ps[t1].append(
                        aux.tile([128, 512], F32, name=f"qp{t1}{fh}",
                                 tag="aux"))
                p_ = qp_ps[t1][fh]
                for kk in (2 * (idx % 4), 2 * (idx % 4) + 1):
                    nc.tensor.matmul(
                        p_[:],
                        lhsT=wq_sb[:, kk, ts(fh, 128)],
                        rhs=qcb[t1][:, kk, :],
                        start=(kk == 0),
                        stop=(kk == KC - 1),
                    )

            pm_hist = {}      # (b, i) -> pm tile
            acc_hist = {}     # b -> accs
            oq_hist = {}      # b -> oq tiles
            TAIL = ((12, 13), (14, 15))
            for b in range(8):
                t1, p = b // 2, b % 2
                accs = [apsum.tile([128, 512], F32, name=f"acc{j}",
                                   tag=f"acc{j}") for j in range(2)]
                acc_hist[b] = accs
                for i in range(T2C):
                    # ---- DMA prefetch points --------------------------------
                    if b == 0 and i == 2:
                        qcb_dma(2)
                    if b == 0 and i == 7:
                        vcb_dma(2)
                    if b == 0 and i == 11:
                        vcb_dma(3)
                    if b == 1 and i == 12:
                        qcb_dma(3)
                    if p == 1 and t1 + 1 < T1C and i == 8:
                        mask_dma(t1 + 1, 0)
                    if p == 0 and b >= 2 and i == 0:
                        mask_dma(t1, 1)
                    # ---- scores first: keeps ACT fed ------------------------
                    psc = scores_mm(b, i)
                    pm_hist[(b, i)] = exp_mask(b, i, psc)
                    # ---- previous block wrap-up -----------------------------
                    if b >= 1 and i < 2:
                        for i2 in TAIL[i]:
                            attnv_mm(b - 1, i2, pm_hist[(b - 1, i2)],
                                     acc_hist[b - 1])
                    if b >= 1 and i == 2:
                        oq_hist[b - 1] = normalize(b - 1, acc_hist[b - 1])
                        del acc_hist[b - 1]
                    if b >= 1 and i == 3:
                        transpose_evac(b - 1, oq_hist.pop(b - 1))
                    # ---- this block's attn@V (lag 4) ------------------------
                    if i >= 4:
                        attnv_mm(b, i - 4, pm_hist[(b, i - 4)], accs)
                    # ---- interleaves ----------------------------------------
                    if b == 0 and i >= 3:
                        vproj(i - 3)
                        if i >= 13:
                            vproj(i)
                    if b == 0 and 2 <= i <= 5:
                        # trailing K-th1 repack, one head per step
                        repack(KT_sb[0], KT_sb[1], KT_hc,
                               slice(1024, 2048), nc.vector, hhs=[i - 2],
                               ssl=slice(1024, 2048))
                    if p == 1 and t1 + 1 < T1C:
                        # Q projection for the next t1, fine-grained
                        nt = t1 + 1
                        if 3 <= i <= 6:
                            qproj_sweep(nt, i - 3)
                        elif 7 <= i <= 10:
                            qproj_sweep(nt, i - 3)
                            if i == 7:
                                qt_rope[nt] = Rope(qp_ps[nt][0], qp_ps[nt][1],
                                                   ts(nt, 512), 512, "q")
                                qt_rope[nt].evac0(False)
                            elif i == 8:
                                qt_rope[nt].muls0()
                        elif i == 11:
                            qt_rope[nt].evac1(False)
                        elif i == 12:
                            qt_rope[nt].muls1()
                        elif i == 13:
                            qt_hist[nt] = [
                                prpool.tile([128, 512], FP16, name=f"QTt{h}",
                                            tag=f"qr{h}") for h in range(2)]
                            qt_rope[nt].combine(qt_hist[nt][0][:],
                                                qt_hist[nt][1][:])
                        elif i in (14, 15):
                            qt = qt_hist[nt]
                            repack(qt[0], qt[1], QT_hc,
                                   slice(nt * 512, (nt + 1) * 512),
                                   nc.vector,
                                   hhs=range(0, 2) if i == 14 else range(2, 4))
                    if p == 0 and b >= 2:
                        # output projection for t1o = t1 - 1 (8 chunks)
                        if i in (4, 5, 6, 7):
                            outproj_chunk(t1 - 1, i - 4)
                        elif i in (8, 10, 12, 14):
                            outproj_chunk(t1 - 1, 4 + (i - 8) // 2)

            # ---- tail: finish block 7, then t1=3 output projection ----------
            b = 7
            for i2 in TAIL[0] + TAIL[1]:
                attnv_mm(b, i2, pm_hist[(b, i2)], acc_hist[b])
            oqs = normalize(b, acc_hist[b])
            transpose_evac(b, oqs)
            for ci in range(8):
                outproj_chunk(3, ci, psc_bank=(ci % 2 == 1))

    nc.compile()
    return nc


def shard_inputs(q, k, v, mask, w_q, w_k, w_v, w_o):
    q = np.asarray(q, np.float32)
    k = np.asarray(k, np.float32)
    v = np.asarray(v, np.float32)
    w_q = np.asarray(w_q, np.float32)
    w_k = np.asarray(w_k, np.float32)
    w_v = np.asarray(w_v, np.float32)
    w_o = np.asarray(w_o, np.float32)
    mask = np.asarray(mask)

    qT = [np.ascontiguousarray(q[b].T).astype(np.float16) for b in range(B)]
    kT = [np.ascontiguousarray(k[b].T).astype(np.float16) for b in range(B)]
    vT = [np.ascontiguousarray(v[b].T).astype(np.float16) for b in range(B)]
    maskT_bf = np.ascontiguousarray(mask[0, 0].T).astype(np.float16)  # [k, q]
    md = maskT_bf.reshape(L, T1C, 512)
    mask_dup = np.ascontiguousarray(
        np.broadcast_to(md[:, :, None, :], (L, T1C, 2, 512))).astype(
            np.float16)

    inv = 1.0 / (10000.0 ** (np.arange(0, DK, 2) / DK))   # [32]
    t = np.arange(L)
    fr = np.outer(inv, t)                                 # [32, 2048]
    cos_tab = np.tile(np.cos(fr), (4, 1)).astype(np.float16)  # [128, 2048]
    sin_tab = np.tile(np.sin(fr), (4, 1)).astype(np.float16)
    ident = np.eye(128, dtype=np.float32)

    even = np.arange(0, DK, 2)
    odd = np.arange(1, DK, 2)
    scale = 1.0 / np.sqrt(DK)

    in_maps = []
    for core in range(N_CORES):
        b, g = divmod(core, N_CORES // B)
        hs = [HG * g + i for i in range(HG)]
        rows_qk = np.concatenate([h * DK + even for h in hs]
                                 + [h * DK + odd for h in hs])
        rows_v = np.concatenate([np.arange(h * DK, (h + 1) * DK) for h in hs])
        in_maps.append({
            "qT": qT[b],
            "kT": kT[b],
            "vT": vT[b],
            "wq": np.ascontiguousarray((w_q[rows_qk, :] * scale).T).astype(np.float16),
            "wk": np.ascontiguousarray(w_k[rows_qk, :].T).astype(np.float16),
            "wv": np.ascontiguousarray(w_v[rows_v, :].T).astype(np.float16),
            "wo": np.ascontiguousarray(w_o[:, rows_v].T).astype(np.float16),
            "cosT": cos_tab,
            "sinT": sin_tab,
            "maskT": mask_dup,
            "ident": ident,
        })
    return in_maps


_compiled = None


def _get_compiled():
    global _compiled
    if _compiled is None:
        _compiled = build_kernel()
    return _compiled


def kernel(q, k, v, mask, w_q, w_k, w_v, w_o, _trace=False, _trace_cores=None):
    from concourse.bass_utils import run_bass_kernel_spmd

    nc = _get_compiled()
    in_maps = shard_inputs(q, k, v, mask, w_q, w_k, w_v, w_o)
    res = run_bass_kernel_spmd(
        nc, in_maps, core_ids=list(range(N_CORES)),
        trace=_trace, trace_cores=_trace_cores,
    )
    out = np.zeros((B, L, D), np.float32)
    for core in range(N_CORES):
        out[core // (N_CORES // B)] += res.results[core]["out"].astype(np.float32)
    kernel._last_results = res
    return out
